# revision 31
# baseline (speedup 1.0000x reference)
"""Single-head dot-product self-attention on 8 Trainium2 NeuronCores.

Problem: x[4,2048,768], Wq/Wk/Wv[768,768] ->
  softmax((x@Wq)(x@Wk)^T / sqrt(768)) @ (x@Wv), all fp32.

Sharding: 8 cores = 4 batches x 2 query-halves. Each core handles 1024
queries over all 2048 keys. The query half is selected host-side by
rotating the sequence so each core's queries are rows 0..1023.

The Q/K chain is reassociated host-side: scores = x (Wq Wk^T) x^T with
Wqk = Wq @ Wk^T precomputed in fp64 -> fp32. This removes the K projection
(-1/6 of PE streaming) and the K AllGather entirely: the full x^T is
already resident per-core, so the scores' stationary operand is just
host-quantized fp8 x^T (in UNROLLED global key order, matching the
pair-rank order the V AllGather produces). On-chip, A^T = Wqk^T x_q^T is
computed in f32r (fp32 operands, 1 cycle/row at N>=512, same stream rate
as bf16) and quantized to fp8; the scores matmul contracts fp8 A^T against
fp8 x^T in DoubleRow perf mode (2 fp8 weights per PE cell, 2 d-chunks of
contraction per matmul). Running the projections from fp32 inputs more
than pays back the fp8 cost: measured 1.57e-2 output error vs the
baseline's 1.75e-2 (gate 2e-2).

V is projected in f32r for its OWN 1024 keys only; the other half arrives
from the pair core (same batch, other half) via a pairwise DRAM AllGather
(~1.5MB bf16) kicked right after the V matmuls so it completes under the
A projection + attention phase (~80us). exp() runs on ScalarE with
1/sqrt(U) folded into the activation input scale; no max-subtraction
(scores are in [-2,2] for this input distribution). A ones-column appended
to V makes the attention-weight row-sums fall out of the AV matmul as
column 768; normalization is one reciprocal + per-row mul. AV runs bf16
(fp8 attention weights/V would blow the error gate). Accumulation is fp32
everywhere.

The iteration is software-pipelined one step ahead: projections (and the
V exchange they feed) for step i+1 are emitted BEFORE attention for step
i. A^T/V/x^T-fp8 tiles are double-buffered so step i+1's writes land while
step i's attention still reads the previous set.

Both 512-query score blocks are emitted before either AV block so the
ScalarE exp of block 0 drains under block 1's score matmuls and the AV
chains never wait on the activation engine.
"""
import numpy as np

import concourse.bacc as bacc
import concourse.tile as tile
from concourse import mybir
from concourse.bass_utils import run_bass_kernel_spmd

B, S, D, U = 4, 2048, 768, 768
P = 128
NQ = S // 2        # queries per core (also: own keys per core)
DC = D // P        # 6 contraction chunks
UC = U // P        # 6 u-tiles
KT = S // P        # 16 key tiles
KTO = KT // 2      # 8 own key tiles
VW = U + 2         # V width: 768 data + ones col + pad col (even moving-dim)
SCALE = 1.0 / float(np.sqrt(U))
PAIRS = [[0, 1], [2, 3], [4, 5], [6, 7]]

f32 = mybir.dt.float32
f32r = mybir.dt.float32r
bf16 = mybir.dt.bfloat16
fp8 = mybir.dt.float8e4
Exp = mybir.ActivationFunctionType.Exp
DR = mybir.MatmulPerfMode.DoubleRow

_CACHE = {}


def _load_inputs(nc, xtp, x8p, wp, xt32, xt8, wqk, wv, tiny_dma=False,
                 in_dt=f32r):
    """Issue input DMAs into fresh tiles from the persistent pools, split
    across the sync and scalar HWDGE queues so two rings run in parallel."""
    xt_sb = xtp.tile([P, DC, NQ], in_dt, tag="xt")
    x8_sb = x8p.tile([P, DC, S], fp8, tag="x8", name="x8_t")

    def load_w(eng, w_dram):
        wt = wp.tile([P, DC, U], in_dt, tag="w")
        if tiny_dma:
            eng.dma_start(wt[:, :, 0:2], w_dram[:].rearrange(
                "(c p) u -> p c u", p=P)[:, :, 0:2])
        else:
            eng.dma_start(wt[:], w_dram[:].rearrange("(c p) u -> p c u", p=P))
        return wt

    wqk_sb = load_w(nc.scalar, wqk)
    for c in range(DC):
        if tiny_dma:
            nc.sync.dma_start(xt_sb[:, c, 0:2], xt32[c * P:(c + 1) * P, 0:2])
        else:
            nc.sync.dma_start(xt_sb[:, c, :], xt32[c * P:(c + 1) * P, :])
    wv_sb = load_w(nc.scalar, wv)
    if tiny_dma:
        nc.sync.dma_start(x8_sb[:, :, 0:2], xt8[:].rearrange(
            "(c p) s -> p c s", p=P)[:, :, 0:2])
    else:
        nc.sync.dma_start(x8_sb[:], xt8[:].rearrange("(c p) s -> p c s", p=P))
    return xt_sb, x8_sb, wqk_sb, wv_sb


def _emit_proj(nc, tc, loads, at_sb, v_sb, ones, drp, dedup, attn_dt):
    """Phase 1 for one step: V (own half if dedup) + A projections, with
    the pairwise V AllGather kicked as soon as its inputs are ready."""
    xt_sb, x8_sb, wqk_sb, wv_sb = loads
    with (
        tc.tile_pool(name="pjp", bufs=2, space="PSUM") as pjp,
        tc.tile_pool(name="vpsp", bufs=2, space="PSUM") as vpsp,
    ):
        nkt = KTO if dedup else KT

        # V[k,u] = x Wv (own keys), plus ones/pad columns at u=768,769
        for kt_i in range(nkt):
            ps = vpsp.tile([P, U], f32, tag="vps")
            for c in range(DC):
                nc.tensor.matmul(
                    ps[:, 0:512],
                    xt_sb[:, c, kt_i * P:(kt_i + 1) * P],
                    wv_sb[:, c, 0:512],
                    start=(c == 0), stop=(c == DC - 1),
                )
                nc.tensor.matmul(
                    ps[:, 512:768],
                    xt_sb[:, c, kt_i * P:(kt_i + 1) * P],
                    wv_sb[:, c, 512:768],
                    start=(c == 0), stop=(c == DC - 1),
                )
            nc.vector.tensor_copy(v_sb[:, kt_i, 0:U], ps[:])
            nc.vector.tensor_copy(v_sb[:, kt_i, U:VW], ones[:])
        if dedup:
            vob = drp.tile([P, KTO * VW], attn_dt, tag="vob")
            vgb = drp.tile([2, P, KTO * VW], attn_dt, tag="vgb")
            nc.scalar.dma_start(
                vob[:].rearrange("p (t w) -> p t w", w=VW),
                v_sb[:, 0:KTO, :])
            nc.gpsimd.collective_compute(
                "AllGather", mybir.AluOpType.bypass,
                replica_groups=PAIRS, ins=[vob.opt()], outs=[vgb.opt()],
            )

        # A^T[e,q] = Wqk^T x^T (queries = the NQ columns of x_q^T). Each
        # stationary Wqk chunk serves both 512-query blocks back-to-back;
        # both accumulate into one 2-bank PSUM tile so the PSUM->SBUF copy
        # pays the DVE fixed overhead once per u-tile.
        for uc in range(UC):
            ps = pjp.tile([P, NQ], f32, tag="pj")
            for c in range(DC):
                for qb in range(NQ // 512):
                    nc.tensor.matmul(
                        ps[:, qb * 512:(qb + 1) * 512],
                        wqk_sb[:, c, uc * P:(uc + 1) * P],
                        xt_sb[:, c, qb * 512:(qb + 1) * 512],
                        start=(c == 0), stop=(c == DC - 1),
                    )
            nc.vector.tensor_copy(at_sb[:, uc, :], ps[:])

        if dedup:
            # Gathered halves land in pair-rank order == global key order
            # (rank r owns global rows r*NQ..), which matches the unrolled
            # xt8 key order used by the scores matmul. The own half is
            # rewritten with identical bytes; the in-DMAs wait on the
            # bounce-out reads via WAR tracking.
            for r in range(2):
                nc.scalar.dma_start(
                    v_sb[:, r * KTO:(r + 1) * KTO, :],
                    vgb[r].rearrange("p (t w) -> p t w", w=VW),
                )


def _emit_attn(nc, tc, out, at_sb, v_sb, x8_sb, expp, outp, recp, attn_dt,
               av_off=False):
    """Phase 2 for one step: scores^T -> exp -> AV -> normalize -> out."""
    with (
        tc.tile_pool(name="scp", bufs=2, space="PSUM") as scp,
        tc.tile_pool(name="avp", bufs=2, space="PSUM") as avp,
    ):
        ex = expp.tile([P, KT, NQ], attn_dt, tag="exp")

        def av_pair(ps, k, q0, start, stop):
            nc.tensor.matmul(
                ps[:, 0:512], ex[:, k, q0:q0 + P], v_sb[:, k, 0:512],
                start=start, stop=stop,
            )
            nc.tensor.matmul(
                ps[:, 512:VW], ex[:, k, q0:q0 + P], v_sb[:, k, 512:VW],
                start=start, stop=stop,
            )

        # fp8 DoubleRow: 2 d-chunks (K=256) per matmul at 2 MACs/cell/
        # cycle. Each stationary x^T pair serves BOTH 512-query blocks
        # back-to-back, halving LDWEIGHTS traffic (DR weight loads stream
        # 256 columns and are the scores bottleneck otherwise). Both query
        # blocks accumulate into one 2-bank PSUM tile so each key tile
        # needs a single exp: 16 ACT instructions/step instead of 32 keeps
        # ScalarE (1 elem/cycle/lane + 352-cycle fixed cost) mostly behind
        # the PE. At DoubleRow rates the scores matmuls alone are slightly
        # cheaper than the exp stream, so the first AV chain's MM-pairs are
        # interleaved between scores chains (their exp dependency trails by
        # 2 key tiles) to keep the PE fed instead of stalling on the scp
        # WAR against exp.
        av0 = (avp.tile([P, VW], f32, tag="av", name="av0_t")
               if not av_off else None)
        for kt_i in range(KT):
            ps = scp.tile([P, NQ], f32, tag="sc")
            for g in range(DC // 2):
                for qb in range(NQ // 512):
                    nc.tensor.matmul(
                        ps[:, qb * 512:(qb + 1) * 512],
                        x8_sb[:, 2 * g:2 * g + 2,
                              kt_i * P:(kt_i + 1) * P],
                        at_sb[:, 2 * g:2 * g + 2,
                              qb * 512:(qb + 1) * 512],
                        start=(g == 0), stop=(g == DC // 2 - 1),
                        perf_mode=DR,
                    )
            nc.scalar.activation(ex[:, kt_i, :], ps[:], Exp, scale=SCALE)
            if av0 is not None and kt_i >= 2:
                av_pair(av0, kt_i - 2, 0, start=(kt_i == 2), stop=False)

        if av_off:
            return
        for qb in range(NQ // 512):
            # out[q,u] = attn^T.T @ V'; col 768 = attn row-sums
            for qt_i in range(4):
                q0 = qb * 512 + qt_i * P
                if qb == 0 and qt_i == 0:
                    ps = av0
                    for k in range(KT - 2, KT):
                        av_pair(ps, k, q0, start=False, stop=(k == KT - 1))
                else:
                    ps = avp.tile([P, VW], f32, tag="av")
                    for k in range(KT):
                        av_pair(ps, k, q0, start=(k == 0), stop=(k == KT - 1))
                rec = recp.tile([P, 1], f32, tag="rec")
                nc.vector.reciprocal(rec[:], ps[:, U:U + 1])
                ot = outp.tile([P, U], f32, tag="out")
                nc.vector.tensor_scalar_mul(ot[:], ps[:, 0:U], rec[:])
                row = qb * 512 + qt_i * P
                nc.sync.dma_start(out[row:row + P, :], ot[:])


def _build(reps=1, tiny_dma=False, attn_dt=bf16, dedup=True, attn_off=False,
           av_off=False, in_dt=f32r):
    nc = bacc.Bacc("TRN2", target_bir_lowering=False, debug=False)
    xt32 = nc.declare_dram_parameter("xt32", [D, NQ], in_dt, isOutput=False)
    xt8 = nc.declare_dram_parameter("xt8", [D, S], fp8, isOutput=False)
    wqk = nc.declare_dram_parameter("wqk", [D, U], in_dt, isOutput=False)
    wv = nc.declare_dram_parameter("wv", [D, U], in_dt, isOutput=False)
    out = nc.declare_dram_parameter("out", [NQ, U], f32, isOutput=True)
    R = reps if isinstance(reps, int) else len(reps)

    with tile.TileContext(nc) as tc:
        with (
            tc.tile_pool(name="atp", bufs=2) as atp,
            tc.tile_pool(name="vp", bufs=2) as vp,
            tc.tile_pool(name="onep", bufs=1) as onep,
            tc.tile_pool(name="xtp", bufs=1) as xtp,
            tc.tile_pool(name="x8p", bufs=2) as x8p,
            tc.tile_pool(name="wp", bufs=2) as wp,
            tc.tile_pool(name="drp", bufs=2, space="DRAM") as drp,
            # bufs=1: exp(i+1) writes can't race AV(i) reads — the PE is
            # serial, so scores(i+1) (which feeds exp) follows AV(i).
            tc.tile_pool(name="expp", bufs=1) as expp,
            tc.tile_pool(name="outp", bufs=3) as outp,
            tc.tile_pool(name="recp", bufs=4) as recp,
        ):
            ones = onep.tile([P, 2], f32)
            nc.vector.memset(ones[:], 1.0)

            # Warm the PE's HAM clock gate once per dispatch, during the
            # first step's input-DMA window.
            with tc.tile_pool(name="wrm", bufs=1, space="PSUM") as wrmp:
                wrm = wrmp.tile([2, 2], f32)
                for _w in range(40):
                    nc.tensor.matmul(wrm[:], ones[:], ones[:],
                                     start=True, stop=True)

            def tiles():
                at_sb = atp.tile([P, DC, NQ], fp8, tag="at", name="at_t")
                v_sb = vp.tile([P, KT, VW], attn_dt, tag="v", name="v_t")
                return at_sb, v_sb

            # Software pipeline: P1(1); for i: [P1(i+1); load(i+2); P2(i)]
            loads = _load_inputs(nc, xtp, x8p, wp, xt32, xt8, wqk, wv,
                                 tiny_dma, in_dt)
            cur = tiles()
            cur_x8 = loads[1]
            _emit_proj(nc, tc, loads, *cur, ones, drp, dedup, attn_dt)
            if R >= 2:
                loads = _load_inputs(nc, xtp, x8p, wp, xt32, xt8, wqk, wv,
                                     tiny_dma, in_dt)
            for i in range(1, R + 1):
                if i + 1 <= R:
                    nxt = tiles()
                    nxt_x8 = loads[1]
                    _emit_proj(nc, tc, loads, *nxt, ones, drp, dedup,
                               attn_dt)
                if i + 2 <= R:
                    loads = _load_inputs(nc, xtp, x8p, wp, xt32, xt8, wqk,
                                         wv, tiny_dma, in_dt)
                if not attn_off:
                    _emit_attn(nc, tc, out, *cur, cur_x8, expp, outp, recp,
                               attn_dt, av_off)
                if i + 1 <= R:
                    cur = nxt
                    cur_x8 = nxt_x8

    nc.finalize()
    return nc


def _get_nc():
    if "nc" not in _CACHE:
        _CACHE["nc"] = _build()
    return _CACHE["nc"]


def _fp8(a):
    return np.ascontiguousarray(a.astype(mybir.dt.np(fp8)))


def _make_in_maps(x, Wq, Wk, Wv):
    x = np.asarray(x, dtype=np.float32)
    Wqk = np.ascontiguousarray(
        (np.asarray(Wq, np.float64) @ np.asarray(Wk, np.float64).T)
        .astype(np.float32))
    Wv32 = np.ascontiguousarray(np.asarray(Wv, dtype=np.float32))
    in_maps = []
    for c in range(8):
        b, h = divmod(c, 2)
        xb = np.roll(x[b], -h * NQ, axis=0)  # this core's queries -> rows 0..NQ-1
        in_maps.append({
            "xt32": np.ascontiguousarray(xb.T[:, :NQ]),
            "xt8": _fp8(x[b].T),             # keys in global (unrolled) order
            "wqk": Wqk, "wv": Wv32,
        })
    return in_maps


def kernel(x, Wq, Wk, Wv):
    nc = _get_nc()
    in_maps = _make_in_maps(x, Wq, Wk, Wv)
    res = run_bass_kernel_spmd(nc, in_maps, core_ids=list(range(8)))
    out = np.empty((B, S, U), np.float32)
    for c in range(8):
        b, h = divmod(c, 2)
        out[b, h * NQ:(h + 1) * NQ] = res.results[c]["out"]
    return out


# revision 32
# speedup vs baseline: 1.0306x; 1.0306x over previous
"""Single-head dot-product self-attention on 8 Trainium2 NeuronCores.

Problem: x[4,2048,768], Wq/Wk/Wv[768,768] ->
  softmax((x@Wq)(x@Wk)^T / sqrt(768)) @ (x@Wv), all fp32.

Sharding: 8 cores = 4 batches x 2 query-halves. Each core handles 1024
queries over all 2048 keys. The query half is selected host-side by
rotating the sequence so each core's queries are rows 0..1023.

The Q/K chain is reassociated host-side: scores = x (Wq Wk^T) x^T with
Wqk = Wq @ Wk^T precomputed in fp64 -> fp32. This removes the K projection
(-1/6 of PE streaming) and the K AllGather entirely: the full x^T is
already resident per-core, so the scores' stationary operand is just
host-quantized fp8 x^T (in UNROLLED global key order, matching the
pair-rank order the V AllGather produces). On-chip, A^T = Wqk^T x_q^T is
computed in f32r (fp32 operands, 1 cycle/row at N>=512, same stream rate
as bf16) and quantized to fp8; the scores matmul contracts fp8 A^T against
fp8 x^T in DoubleRow perf mode (2 fp8 weights per PE cell, 2 d-chunks of
contraction per matmul). Running the projections from fp32 inputs more
than pays back the fp8 cost: measured 1.57e-2 output error vs the
baseline's 1.75e-2 (gate 2e-2).

V is projected in f32r for its OWN 1024 keys only; the other half arrives
from the pair core (same batch, other half) via a pairwise DRAM AllGather
(~1.5MB bf16) kicked right after the V matmuls so it completes under the
A projection + attention phase (~80us). exp() runs on ScalarE with
1/sqrt(U) folded into the activation input scale; no max-subtraction
(scores are in [-2,2] for this input distribution). A ones-column appended
to V makes the attention-weight row-sums fall out of the AV matmul as
column 768; normalization is one reciprocal + per-row mul. AV runs bf16
(fp8 attention weights/V would blow the error gate). Accumulation is fp32
everywhere.

The iteration is software-pipelined one step ahead: projections (and the
V exchange they feed) for step i+1 are emitted BEFORE attention for step
i. A^T/V/x^T-fp8 tiles are double-buffered so step i+1's writes land while
step i's attention still reads the previous set.

Both 512-query score blocks accumulate into one 2-bank PSUM tile so each
key tile needs a single [128,1024] exp: 16 ACT instructions per step
instead of 32 keeps ScalarE (1 elem/cycle/lane + 352-cycle fixed cost per
instruction) behind the PE. At DoubleRow rates (~180ns/matmul measured vs
~229 for bf16) the scores matmuls alone undercut the exp stream, so the
first AV chain's MM-pairs are interleaved between scores chains — their
exp dependencies trail two key-tiles behind — keeping the PE fed instead
of stalling on the PSUM-pool WAR against exp.
"""
import numpy as np

import concourse.bacc as bacc
import concourse.tile as tile
from concourse import mybir
from concourse.bass_utils import run_bass_kernel_spmd

B, S, D, U = 4, 2048, 768, 768
P = 128
NQ = S // 2        # queries per core (also: own keys per core)
DC = D // P        # 6 contraction chunks
UC = U // P        # 6 u-tiles
KT = S // P        # 16 key tiles
KTO = KT // 2      # 8 own key tiles
VW = U + 2         # V width: 768 data + ones col + pad col (even moving-dim)
SCALE = 1.0 / float(np.sqrt(U))
PAIRS = [[0, 1], [2, 3], [4, 5], [6, 7]]

f32 = mybir.dt.float32
f32r = mybir.dt.float32r
bf16 = mybir.dt.bfloat16
fp8 = mybir.dt.float8e4
Exp = mybir.ActivationFunctionType.Exp
DR = mybir.MatmulPerfMode.DoubleRow

_CACHE = {}


def _load_inputs(nc, xtp, x8p, wp, xt32, xt8, wqk, wv, tiny_dma=False,
                 in_dt=f32r):
    """Issue input DMAs into fresh tiles from the persistent pools, split
    across the sync and scalar HWDGE queues so two rings run in parallel."""
    xt_sb = xtp.tile([P, DC, NQ], in_dt, tag="xt")
    x8_sb = x8p.tile([P, DC, S], fp8, tag="x8", name="x8_t")

    def load_w(eng, w_dram):
        wt = wp.tile([P, DC, U], in_dt, tag="w")
        if tiny_dma:
            eng.dma_start(wt[:, :, 0:2], w_dram[:].rearrange(
                "(c p) u -> p c u", p=P)[:, :, 0:2])
        else:
            eng.dma_start(wt[:], w_dram[:].rearrange("(c p) u -> p c u", p=P))
        return wt

    wqk_sb = load_w(nc.scalar, wqk)
    for c in range(DC):
        if tiny_dma:
            nc.sync.dma_start(xt_sb[:, c, 0:2], xt32[c * P:(c + 1) * P, 0:2])
        else:
            nc.sync.dma_start(xt_sb[:, c, :], xt32[c * P:(c + 1) * P, :])
    wv_sb = load_w(nc.scalar, wv)
    if tiny_dma:
        nc.sync.dma_start(x8_sb[:, :, 0:2], xt8[:].rearrange(
            "(c p) s -> p c s", p=P)[:, :, 0:2])
    else:
        nc.sync.dma_start(x8_sb[:], xt8[:].rearrange("(c p) s -> p c s", p=P))
    return xt_sb, x8_sb, wqk_sb, wv_sb


def _emit_proj(nc, tc, loads, at_sb, v_sb, ones, drp, dedup, attn_dt):
    """Phase 1 for one step: V (own half if dedup) + A projections, with
    the pairwise V AllGather kicked as soon as its inputs are ready."""
    xt_sb, x8_sb, wqk_sb, wv_sb = loads
    with (
        tc.tile_pool(name="pjp", bufs=2, space="PSUM") as pjp,
        tc.tile_pool(name="vpsp", bufs=2, space="PSUM") as vpsp,
    ):
        nkt = KTO if dedup else KT

        # V[k,u] = x Wv (own keys), plus ones/pad columns at u=768,769
        for kt_i in range(nkt):
            ps = vpsp.tile([P, U], f32, tag="vps")
            for c in range(DC):
                nc.tensor.matmul(
                    ps[:, 0:512],
                    xt_sb[:, c, kt_i * P:(kt_i + 1) * P],
                    wv_sb[:, c, 0:512],
                    start=(c == 0), stop=(c == DC - 1),
                )
                nc.tensor.matmul(
                    ps[:, 512:768],
                    xt_sb[:, c, kt_i * P:(kt_i + 1) * P],
                    wv_sb[:, c, 512:768],
                    start=(c == 0), stop=(c == DC - 1),
                )
            nc.vector.tensor_copy(v_sb[:, kt_i, 0:U], ps[:])
            nc.vector.tensor_copy(v_sb[:, kt_i, U:VW], ones[:])
        if dedup:
            vob = drp.tile([P, KTO * VW], attn_dt, tag="vob")
            vgb = drp.tile([2, P, KTO * VW], attn_dt, tag="vgb")
            nc.scalar.dma_start(
                vob[:].rearrange("p (t w) -> p t w", w=VW),
                v_sb[:, 0:KTO, :])
            nc.gpsimd.collective_compute(
                "AllGather", mybir.AluOpType.bypass,
                replica_groups=PAIRS, ins=[vob.opt()], outs=[vgb.opt()],
            )

        # A^T[e,q] = Wqk^T x^T (queries = the NQ columns of x_q^T). Each
        # stationary Wqk chunk serves both 512-query blocks back-to-back;
        # both accumulate into one 2-bank PSUM tile so the PSUM->SBUF copy
        # pays the DVE fixed overhead once per u-tile.
        for uc in range(UC):
            ps = pjp.tile([P, NQ], f32, tag="pj")
            for c in range(DC):
                for qb in range(NQ // 512):
                    nc.tensor.matmul(
                        ps[:, qb * 512:(qb + 1) * 512],
                        wqk_sb[:, c, uc * P:(uc + 1) * P],
                        xt_sb[:, c, qb * 512:(qb + 1) * 512],
                        start=(c == 0), stop=(c == DC - 1),
                    )
            nc.vector.tensor_copy(at_sb[:, uc, :], ps[:])

        if dedup:
            # Gathered halves land in pair-rank order == global key order
            # (rank r owns global rows r*NQ..), which matches the unrolled
            # xt8 key order used by the scores matmul. The own half is
            # rewritten with identical bytes; the in-DMAs wait on the
            # bounce-out reads via WAR tracking.
            for r in range(2):
                nc.scalar.dma_start(
                    v_sb[:, r * KTO:(r + 1) * KTO, :],
                    vgb[r].rearrange("p (t w) -> p t w", w=VW),
                )


def _emit_attn(nc, tc, out, at_sb, v_sb, x8_sb, expp, outp, recp, attn_dt,
               av_off=False):
    """Phase 2 for one step: scores^T -> exp -> AV -> normalize -> out."""
    with (
        tc.tile_pool(name="scp", bufs=2, space="PSUM") as scp,
        tc.tile_pool(name="avp", bufs=2, space="PSUM") as avp,
    ):
        ex = expp.tile([P, KT, NQ], attn_dt, tag="exp")

        def av_pair(ps, k, q0, start, stop):
            nc.tensor.matmul(
                ps[:, 0:512], ex[:, k, q0:q0 + P], v_sb[:, k, 0:512],
                start=start, stop=stop,
            )
            nc.tensor.matmul(
                ps[:, 512:VW], ex[:, k, q0:q0 + P], v_sb[:, k, 512:VW],
                start=start, stop=stop,
            )

        # fp8 DoubleRow: 2 d-chunks (K=256) per matmul at 2 MACs/cell/
        # cycle. Each stationary x^T pair serves BOTH 512-query blocks
        # back-to-back, halving LDWEIGHTS traffic (DR weight loads stream
        # 256 columns and are the scores bottleneck otherwise). Both query
        # blocks accumulate into one 2-bank PSUM tile so each key tile
        # needs a single exp: 16 ACT instructions/step instead of 32 keeps
        # ScalarE (1 elem/cycle/lane + 352-cycle fixed cost) mostly behind
        # the PE. At DoubleRow rates the scores matmuls alone are slightly
        # cheaper than the exp stream, so the first AV chain's MM-pairs are
        # interleaved between scores chains (their exp dependency trails by
        # 2 key tiles) to keep the PE fed instead of stalling on the scp
        # WAR against exp.
        av0 = (avp.tile([P, VW], f32, tag="av", name="av0_t")
               if not av_off else None)
        for kt_i in range(KT):
            ps = scp.tile([P, NQ], f32, tag="sc")
            for g in range(DC // 2):
                for qb in range(NQ // 512):
                    nc.tensor.matmul(
                        ps[:, qb * 512:(qb + 1) * 512],
                        x8_sb[:, 2 * g:2 * g + 2,
                              kt_i * P:(kt_i + 1) * P],
                        at_sb[:, 2 * g:2 * g + 2,
                              qb * 512:(qb + 1) * 512],
                        start=(g == 0), stop=(g == DC // 2 - 1),
                        perf_mode=DR,
                    )
            nc.scalar.activation(ex[:, kt_i, :], ps[:], Exp, scale=SCALE)
            if av0 is not None and kt_i >= 2:
                av_pair(av0, kt_i - 2, 0, start=(kt_i == 2), stop=False)

        if av_off:
            return
        for qb in range(NQ // 512):
            # out[q,u] = attn^T.T @ V'; col 768 = attn row-sums
            for qt_i in range(4):
                q0 = qb * 512 + qt_i * P
                if qb == 0 and qt_i == 0:
                    ps = av0
                    for k in range(KT - 2, KT):
                        av_pair(ps, k, q0, start=False, stop=(k == KT - 1))
                else:
                    ps = avp.tile([P, VW], f32, tag="av")
                    for k in range(KT):
                        av_pair(ps, k, q0, start=(k == 0), stop=(k == KT - 1))
                rec = recp.tile([P, 1], f32, tag="rec")
                nc.vector.reciprocal(rec[:], ps[:, U:U + 1])
                ot = outp.tile([P, U], f32, tag="out")
                nc.vector.tensor_scalar_mul(ot[:], ps[:, 0:U], rec[:])
                row = qb * 512 + qt_i * P
                nc.sync.dma_start(out[row:row + P, :], ot[:])


def _build(reps=1, tiny_dma=False, attn_dt=bf16, dedup=True, attn_off=False,
           av_off=False, in_dt=f32r):
    nc = bacc.Bacc("TRN2", target_bir_lowering=False, debug=False)
    xt32 = nc.declare_dram_parameter("xt32", [D, NQ], in_dt, isOutput=False)
    xt8 = nc.declare_dram_parameter("xt8", [D, S], fp8, isOutput=False)
    wqk = nc.declare_dram_parameter("wqk", [D, U], in_dt, isOutput=False)
    wv = nc.declare_dram_parameter("wv", [D, U], in_dt, isOutput=False)
    out = nc.declare_dram_parameter("out", [NQ, U], f32, isOutput=True)
    R = reps if isinstance(reps, int) else len(reps)

    with tile.TileContext(nc) as tc:
        with (
            tc.tile_pool(name="atp", bufs=2) as atp,
            tc.tile_pool(name="vp", bufs=2) as vp,
            tc.tile_pool(name="onep", bufs=1) as onep,
            tc.tile_pool(name="xtp", bufs=1) as xtp,
            tc.tile_pool(name="x8p", bufs=2) as x8p,
            tc.tile_pool(name="wp", bufs=2) as wp,
            tc.tile_pool(name="drp", bufs=2, space="DRAM") as drp,
            # bufs=1: exp(i+1) writes can't race AV(i) reads — the PE is
            # serial, so scores(i+1) (which feeds exp) follows AV(i).
            tc.tile_pool(name="expp", bufs=1) as expp,
            tc.tile_pool(name="outp", bufs=3) as outp,
            tc.tile_pool(name="recp", bufs=4) as recp,
        ):
            ones = onep.tile([P, 2], f32)
            nc.vector.memset(ones[:], 1.0)

            # Warm the PE's HAM clock gate once per dispatch, during the
            # first step's input-DMA window.
            with tc.tile_pool(name="wrm", bufs=1, space="PSUM") as wrmp:
                wrm = wrmp.tile([2, 2], f32)
                for _w in range(40):
                    nc.tensor.matmul(wrm[:], ones[:], ones[:],
                                     start=True, stop=True)

            def tiles():
                at_sb = atp.tile([P, DC, NQ], fp8, tag="at", name="at_t")
                v_sb = vp.tile([P, KT, VW], attn_dt, tag="v", name="v_t")
                return at_sb, v_sb

            # Software pipeline: P1(1); for i: [P1(i+1); load(i+2); P2(i)]
            loads = _load_inputs(nc, xtp, x8p, wp, xt32, xt8, wqk, wv,
                                 tiny_dma, in_dt)
            cur = tiles()
            cur_x8 = loads[1]
            _emit_proj(nc, tc, loads, *cur, ones, drp, dedup, attn_dt)
            if R >= 2:
                loads = _load_inputs(nc, xtp, x8p, wp, xt32, xt8, wqk, wv,
                                     tiny_dma, in_dt)
            for i in range(1, R + 1):
                if i + 1 <= R:
                    nxt = tiles()
                    nxt_x8 = loads[1]
                    _emit_proj(nc, tc, loads, *nxt, ones, drp, dedup,
                               attn_dt)
                if i + 2 <= R:
                    loads = _load_inputs(nc, xtp, x8p, wp, xt32, xt8, wqk,
                                         wv, tiny_dma, in_dt)
                if not attn_off:
                    _emit_attn(nc, tc, out, *cur, cur_x8, expp, outp, recp,
                               attn_dt, av_off)
                if i + 1 <= R:
                    cur = nxt
                    cur_x8 = nxt_x8

    nc.finalize()
    return nc


def _get_nc():
    if "nc" not in _CACHE:
        _CACHE["nc"] = _build()
    return _CACHE["nc"]


def _fp8(a):
    return np.ascontiguousarray(a.astype(mybir.dt.np(fp8)))


def _make_in_maps(x, Wq, Wk, Wv):
    x = np.asarray(x, dtype=np.float32)
    Wqk = np.ascontiguousarray(
        (np.asarray(Wq, np.float64) @ np.asarray(Wk, np.float64).T)
        .astype(np.float32))
    Wv32 = np.ascontiguousarray(np.asarray(Wv, dtype=np.float32))
    in_maps = []
    for c in range(8):
        b, h = divmod(c, 2)
        xb = np.roll(x[b], -h * NQ, axis=0)  # this core's queries -> rows 0..NQ-1
        in_maps.append({
            "xt32": np.ascontiguousarray(xb.T[:, :NQ]),
            "xt8": _fp8(x[b].T),             # keys in global (unrolled) order
            "wqk": Wqk, "wv": Wv32,
        })
    return in_maps


def kernel(x, Wq, Wk, Wv):
    nc = _get_nc()
    in_maps = _make_in_maps(x, Wq, Wk, Wv)
    res = run_bass_kernel_spmd(nc, in_maps, core_ids=list(range(8)))
    out = np.empty((B, S, U), np.float32)
    for c in range(8):
        b, h = divmod(c, 2)
        out[b, h * NQ:(h + 1) * NQ] = res.results[c]["out"]
    return out
